# revision 4
# baseline (speedup 1.0000x reference)
"""CenterLoss kernel for Trainium2 (8 NeuronCores, Bass/Tile).

Strategy (class-sharded):
  - centers [100000,128] split into 8 shards of 12500 rows (+1 junk row).
  - Batch items routed on host to the core owning their class, sorted by
    class, packed into 128-item chunks such that no class's run crosses a
    chunk boundary (pad with junk items). All host work is integer index
    bookkeeping on y only.
  - Per core the device:
      * bulk-copies its centers shard to the output (dominant HBM traffic)
      * streams item chunks: indirect-gather center rows, diff = c - x,
        loss += sum(diff^2), one-hot(rank) matmul merges duplicate classes
        within a chunk and produces per-class counts, new row =
        c - alpha/(count+1) * upd, indirect-scatter rows to the output.
  - Host concatenates the 8 output shards and sums the 8 loss partials.
"""

import numpy as np

import concourse.bass as bass
import concourse.tile as tile
from concourse import bacc, mybir
from concourse import bass_utils

NB_CLASS = 100000
DIM = 128
BATCH = 16384
LOSS_WEIGHT = 0.01
ALPHA = 0.05

NCORES = 8
SHARD = NB_CLASS // NCORES  # 12500
JUNK = SHARD  # junk row index in the per-core shard (extra row)
P = 128  # chunk size == partitions
NPAD = 2560  # padded items per core
NCHUNK = NPAD // P  # 20
GRP = 5  # chunks per indirect-DMA group
NGRP = NCHUNK // GRP  # 4
COPY_SLICES = 4  # bulk-copy split across HWDGE queues

FP = mybir.dt.float32
I32 = mybir.dt.int32


def _build_program():
    nc = bacc.Bacc("TRN2", target_bir_lowering=False, debug=False,
                   num_devices=NCORES)

    centers_t = nc.dram_tensor("centers_s", [SHARD + 1, DIM], FP,
                               kind="ExternalInput")
    x_t = nc.dram_tensor("x_s", [NPAD, DIM], FP, kind="ExternalInput")
    lidx_t = nc.dram_tensor("lidx_s", [P, NCHUNK], I32, kind="ExternalInput")
    rank_t = nc.dram_tensor("rank_s", [P, NCHUNK], FP, kind="ExternalInput")
    uslot_t = nc.dram_tensor("uslot_s", [P, NCHUNK], I32, kind="ExternalInput")

    newc_t = nc.dram_tensor("newc_s", [SHARD + 1, DIM], FP,
                            kind="ExternalOutput")
    loss_t = nc.dram_tensor("loss_s", [1, 1], FP, kind="ExternalOutput")

    centers_ap = centers_t.ap()
    newc_ap = newc_t.ap()

    with tile.TileContext(nc) as tc:
        with tc.tile_pool(name="const", bufs=1) as cpool, \
             tc.tile_pool(name="meta", bufs=1) as mpool, \
             tc.tile_pool(name="big", bufs=3) as bpool, \
             tc.tile_pool(name="work", bufs=6) as wpool, \
             tc.tile_pool(name="psum", bufs=4, space="PSUM") as ppool, \
             tc.tile_pool(name="psl", bufs=1, space="PSUM") as plpool:

            # ---- one-time constants ----
            iota_i = cpool.tile([P, P], I32)
            nc.gpsimd.iota(iota_i[:], pattern=[[1, P]], base=0,
                           channel_multiplier=0)
            iota_f = cpool.tile([P, P], FP)
            nc.vector.tensor_copy(iota_f[:], iota_i[:])
            ones_col = cpool.tile([P, 1], FP)
            nc.vector.memset(ones_col[:], 1.0)
            lacc = cpool.tile([P, 1], FP)
            nc.vector.memset(lacc[:], 0.0)

            # ---- bulk copy centers shard -> output shard (DRAM->DRAM) ----
            rows = SHARD // COPY_SLICES
            for s in range(COPY_SLICES):
                eng = nc.sync if s % 2 == 0 else nc.scalar
                r0, r1 = s * rows, (s + 1) * rows
                eng.dma_start(newc_ap[r0:r1, :], centers_ap[r0:r1, :])

            # ---- metadata (single DMAs) ----
            lidx_m = mpool.tile([P, NCHUNK], I32)
            nc.sync.dma_start(lidx_m[:], lidx_t.ap()[:, :])
            rank_m = mpool.tile([P, NCHUNK], FP)
            nc.scalar.dma_start(rank_m[:], rank_t.ap()[:, :])
            uslot_m = mpool.tile([P, NCHUNK], I32)
            nc.sync.dma_start(uslot_m[:], uslot_t.ap()[:, :])

            # ---- item-chunk pipeline ----
            for g in range(NGRP):
                c0 = g * GRP
                xg = bpool.tile([P, GRP, DIM], FP, tag="xg")
                eng = nc.sync if g % 2 == 0 else nc.scalar
                eng.dma_start(
                    xg[:],
                    x_t.ap()[c0 * P:(c0 + GRP) * P, :]
                       .rearrange("(c p) d -> p c d", p=P))

                for cc in range(GRP):
                    c = c0 + cc
                    # gather this chunk's center rows ([128,1] offsets only:
                    # multi-dim offset APs mis-execute on real HW)
                    cg = wpool.tile([P, DIM], FP, tag="cg")
                    nc.gpsimd.indirect_dma_start(
                        out=cg[:],
                        out_offset=None,
                        in_=centers_ap[:, :],
                        in_offset=bass.IndirectOffsetOnAxis(
                            ap=lidx_m[:, c:c + 1], axis=0))

                    # diff (+ ones column for the counts matmul)
                    diffx = wpool.tile([P, DIM + 1], FP, tag="diffx")
                    nc.vector.tensor_sub(diffx[:, :DIM], cg[:], xg[:, cc, :])
                    nc.vector.memset(diffx[:, DIM:DIM + 1], 1.0)

                    # loss partial: lacc += rowsum(diff^2)
                    # (tensor_tensor_reduce crashes HW on this path; use
                    # mult + reduce)
                    sq = wpool.tile([P, DIM], FP, tag="sq")
                    sacc = wpool.tile([P, 1], FP, tag="sacc")
                    nc.vector.tensor_tensor(
                        out=sq[:], in0=diffx[:, :DIM], in1=diffx[:, :DIM],
                        op=mybir.AluOpType.mult)
                    nc.vector.tensor_reduce(
                        out=sacc[:], in_=sq[:], axis=mybir.AxisListType.X,
                        op=mybir.AluOpType.add)
                    nc.vector.tensor_add(lacc[:], lacc[:], sacc[:])

                    # one-hot of first-occurrence rank
                    onehot = wpool.tile([P, P], FP, tag="onehot")
                    nc.vector.tensor_tensor(
                        out=onehot[:],
                        in0=rank_m[:, c:c + 1].to_broadcast([P, P]),
                        in1=iota_f[:],
                        op=mybir.AluOpType.is_equal)

                    # upd[slot, :DIM] (+ count in col DIM)
                    ps = ppool.tile([P, DIM + 1], FP, tag="ps")
                    nc.tensor.matmul(out=ps[:], lhsT=onehot[:], rhs=diffx[:],
                                     start=True, stop=True)

                    # a = -alpha / (count + 1)
                    n_t = wpool.tile([P, 1], FP, tag="n_t")
                    nc.vector.tensor_scalar_add(n_t[:], ps[:, DIM:DIM + 1], 1.0)
                    rec = wpool.tile([P, 1], FP, tag="rec")
                    nc.vector.reciprocal(rec[:], n_t[:])
                    a_t = wpool.tile([P, 1], FP, tag="a_t")
                    nc.vector.tensor_scalar_mul(a_t[:], rec[:], -ALPHA)

                    # new row = c + a * upd
                    outc = wpool.tile([P, DIM], FP, tag="outc")
                    nc.vector.tensor_scalar(
                        out=outc[:], in0=ps[:, :DIM],
                        scalar1=a_t[:, :1], scalar2=None,
                        op0=mybir.AluOpType.mult)
                    nc.vector.tensor_add(outc[:], outc[:], cg[:])

                    # scatter final rows (unique targets; dups -> junk row).
                    # bounds_check + oob_is_err=False is required: the
                    # no-bounds-regs indirect-scatter ucode wedges on HW.
                    nc.gpsimd.indirect_dma_start(
                        out=newc_ap[:, :],
                        out_offset=bass.IndirectOffsetOnAxis(
                            ap=uslot_m[:, c:c + 1], axis=0),
                        in_=outc[:],
                        in_offset=None,
                        bounds_check=SHARD,
                        oob_is_err=False)

            # ---- loss: cross-partition sum via ones matmul, then scale ----
            psl = plpool.tile([1, 1], FP)
            nc.tensor.matmul(out=psl[:], lhsT=lacc[:], rhs=ones_col[:],
                             start=True, stop=True)
            loss_sb = cpool.tile([1, 1], FP)
            nc.vector.tensor_scalar_mul(loss_sb[:], psl[:],
                                        LOSS_WEIGHT / BATCH)
            nc.sync.dma_start(loss_t.ap()[:, :], loss_sb[:])

    nc.compile()
    return nc


_NC = None


def _get_program():
    global _NC
    if _NC is None:
        _NC = _build_program()
    return _NC


def _pack_core(cls_loc: np.ndarray, x_core: np.ndarray):
    """Pack one core's sorted items into chunks of P with no class run
    crossing a chunk boundary. Returns device input arrays."""
    m = cls_loc.shape[0]
    if m == 0:
        starts = np.zeros(0, np.int64)
        lens = np.zeros(0, np.int64)
    else:
        starts = np.flatnonzero(np.r_[True, cls_loc[1:] != cls_loc[:-1]])
        lens = np.diff(np.r_[starts, m])

    place = np.empty(len(starts), np.int64)
    pos = 0
    for i, L in enumerate(lens):
        room = P - (pos % P)
        if L > room:
            pos += room
        assert L <= P, f"class run of length {L} exceeds chunk size"
        place[i] = pos
        pos += L
    assert pos <= NPAD, f"core needs {pos} slots > NPAD={NPAD}"

    # per-item output position
    out_pos = np.repeat(place, lens) + (np.arange(m) - np.repeat(starts, lens))

    xk = np.zeros((NPAD, DIM), np.float32)
    xk[out_pos] = x_core
    lidx = np.full(NPAD, JUNK, np.int32)
    lidx[out_pos] = cls_loc
    rank = (np.arange(NPAD) % P).astype(np.float32)
    rank[out_pos] = np.repeat((place % P).astype(np.float32), lens)
    uslot = np.full(NPAD, JUNK, np.int32)
    uslot[place] = cls_loc[starts]

    return {
        "x_s": xk,
        "lidx_s": np.ascontiguousarray(lidx.reshape(NCHUNK, P).T),
        "rank_s": np.ascontiguousarray(rank.reshape(NCHUNK, P).T.astype(np.float32)),
        "uslot_s": np.ascontiguousarray(uslot.reshape(NCHUNK, P).T),
    }


def make_in_maps(x: np.ndarray, y: np.ndarray, centers: np.ndarray):
    order = np.argsort(y, kind="stable")
    ys = y[order]
    xs = x[order]
    bounds = np.searchsorted(ys, np.arange(NCORES + 1) * SHARD)

    in_maps = []
    for k in range(NCORES):
        lo, hi = bounds[k], bounds[k + 1]
        im = _pack_core((ys[lo:hi] - k * SHARD).astype(np.int64), xs[lo:hi])
        shard = np.empty((SHARD + 1, DIM), np.float32)
        shard[:SHARD] = centers[k * SHARD:(k + 1) * SHARD]
        shard[SHARD] = 0.0
        im["centers_s"] = shard
        in_maps.append(im)
    return in_maps


LAST_RESULTS = None


def kernel(x: np.ndarray, y: np.ndarray, centers: np.ndarray):
    global LAST_RESULTS
    x = np.ascontiguousarray(np.asarray(x, np.float32))
    y = np.asarray(y, np.int32)
    centers = np.ascontiguousarray(np.asarray(centers, np.float32))

    in_maps = make_in_maps(x, y, centers)
    nc = _get_program()
    res = bass_utils.run_bass_kernel_spmd(nc, in_maps,
                                          core_ids=list(range(NCORES)))
    LAST_RESULTS = res

    new_centers = np.concatenate(
        [res.results[k]["newc_s"][:SHARD] for k in range(NCORES)], axis=0)
    loss = np.float32(sum(float(res.results[k]["loss_s"][0, 0])
                          for k in range(NCORES)))
    return loss, new_centers
